# revision 9
# baseline (speedup 1.0000x reference)
"""Trainium2 Bass kernel for nn_Event_Critic_Net (dual-branch GAT critic).

Math: the reference only reads the GAT output at the LAST node of each
graph (graphs are 32 contiguous nodes), so only edges whose dst is a
graph's last node contribute.  For those edges the softmax-weighted
aggregation commutes with the linear projection W:

    out_g = sigmoid( (sum_n alpha[n] * x[n,:]) @ W + bias )
    alpha[n] = cnt[n]*exp(e[n]) / (sum_n cnt[n]*exp(e[n]) + 1e-16)
    e[n] = leaky_relu(x[n]. w_src + x[last(g)]. w_dst),  w_* = W @ att_*

cnt[n] = number of edges (n -> last(g(n))).  Graph-structure prep
(edge counts, tiling, weight replication) happens on host; all FLOPs
on device.  Sharding: graphs are data-parallel across the 8 cores
(core c owns graphs [c*512, (c+1)*512) == nodes [c*16384, (c+1)*16384)).

Data path is bf16 (PSUM accumulation fp32); softmax/normalization
scalars stay fp32.
"""

import numpy as np
from contextlib import ExitStack

NC = 8            # cores
N = 131072        # nodes total
G = 4096          # graphs
NPG = 32          # nodes per graph
S = 64            # state size
H = 128           # hidden size
NPC = N // NC     # 16384 nodes per core
GPC = G // NC     # 512 graphs per core
T = NPC // 128    # 128 node-tiles per core
SA = 68           # x columns: 64 features | ones@64 | 3 zero pad (fold align)
CH = 16           # node-tiles per a_src chunk
NCHUNK = T // CH  # 8 chunks

_CACHE = {}


def _build_module():
    import concourse.tile as tile
    from concourse import bacc, mybir
    from concourse.alu_op_type import AluOpType as Alu

    f32 = mybir.dt.float32
    bf16 = mybir.dt.bfloat16
    Act = mybir.ActivationFunctionType
    AxX = mybir.AxisListType.X

    nc = bacc.Bacc("TRN2", target_bir_lowering=False, debug=False,
                   num_devices=NC)

    dram = {}

    def din(name, shape, dt=f32):
        dram[name] = nc.dram_tensor(name, shape, dt, kind="ExternalInput")

    for p in ("u", "d"):
        din(f"{p}_xab", [128, T * SA], bf16)
        din(f"{p}_cnt", [128, T])
        din(f"{p}_xlast", [128, 4 * S], bf16)
        din(f"{p}_wsrc", [128, CH * SA], bf16)
        din(f"{p}_wdst", [128, 4 * S], bf16)
        din(f"{p}_bias", [128, 1])
        din(f"{p}_W", [S, H], bf16)
    din("Qm", [4, 128], bf16)
    din("Bm", [128, 4])
    din("ones64", [1, S])
    din("ident", [128, 128])
    din("mlpW", [H, 1], bf16)
    din("mlpb", [1, 1])
    din("eps", [1, 1])
    out_dram = nc.dram_tensor("out", [1, GPC], f32, kind="ExternalOutput")

    with tile.TileContext(nc) as tc, ExitStack() as ctx:
        const = ctx.enter_context(tc.tile_pool(name="const", bufs=1))
        xp = ctx.enter_context(tc.tile_pool(name="xp", bufs=2))
        wk = ctx.enter_context(tc.tile_pool(name="wk", bufs=2))
        ps1 = ctx.enter_context(tc.tile_pool(name="ps1", bufs=1, space="PSUM"))
        ps2 = ctx.enter_context(tc.tile_pool(name="ps2", bufs=2, space="PSUM"))

        def cload(name, shape, dt=f32):
            t = const.tile(shape, dt, tag=name)
            nc.sync.dma_start(t[:], dram[name].ap())
            return t

        Qm = cload("Qm", [4, 128], bf16)
        Bm = cload("Bm", [128, 4])
        ones64 = cload("ones64", [1, S])
        ident = cload("ident", [128, 128])
        mlpW = cload("mlpW", [H, 1], bf16)
        mlpb = cload("mlpb", [1, 1])
        eps = cload("eps", [1, 1])

        sig = {}
        for p in ("u", "d"):
            wsrc = cload(f"{p}_wsrc", [128, CH * SA], bf16)
            wdst = cload(f"{p}_wdst", [128, 4 * S], bf16)
            Wb = cload(f"{p}_W", [S, H], bf16)
            bias = cload(f"{p}_bias", [128, 1])

            # ---- big x load, chunked for DMA/compute overlap ----
            x = xp.tile([128, T * SA], bf16, tag="x")
            for c in range(NCHUNK):
                sl = slice(c * CH * SA, (c + 1) * CH * SA)
                nc.sync.dma_start(x[:, sl], dram[f"{p}_xab"].ap()[:, sl])
            cnt = wk.tile([128, T], f32, tag="cnt")
            nc.sync.dma_start(cnt[:], dram[f"{p}_cnt"].ap())
            xl = wk.tile([128, 4 * S], bf16, tag="xl")
            nc.sync.dma_start(xl[:], dram[f"{p}_xlast"].ap())

            # ---- a_dst at last nodes: mult+reduce, transpose, broadcast ----
            tmp4 = wk.tile([128, 4 * S], bf16, tag="tmp4")
            nc.vector.tensor_tensor(tmp4[:], xl[:], wdst[:], op=Alu.mult)
            adst = wk.tile([128, 4], f32, tag="adst")
            nc.vector.tensor_reduce(
                adst[:], tmp4[:].rearrange("p (j s) -> p j s", s=S),
                axis=AxX, op=Alu.add)
            tp = ps1.tile([4, 128], f32, tag="tp")
            nc.tensor.transpose(tp[:], adst[:], ident[:])
            adT = wk.tile([4, 128], bf16, tag="adT")
            nc.vector.tensor_copy(adT[:], tp[:])
            adbc = ps1.tile([128, T], f32, tag="adbc")
            nc.tensor.matmul(adbc[:], Qm[:], adT[:], start=True, stop=True)

            # ---- a_src: gpsimd mult -> DVE fold(68->34) -> DVE reduce ----
            asrc = wk.tile([128, T], f32, tag="asrc")
            for c in range(NCHUNK):
                csl = slice(c * CH * SA, (c + 1) * CH * SA)
                tmp = wk.tile([128, CH * SA], bf16, tag="tmp")
                nc.gpsimd.tensor_tensor(tmp[:], x[:, csl], wsrc[:], op=Alu.mult)
                t3 = tmp[:].rearrange("p (i s) -> p i s", s=SA)
                u = wk.tile([128, CH * (SA // 2)], bf16, tag="u")
                u3 = u[:].rearrange("p (i s) -> p i s", s=SA // 2)
                nc.vector.tensor_tensor(
                    u3[:, :, :], t3[:, :, 0:SA // 2], t3[:, :, SA // 2:SA],
                    op=Alu.add)
                nc.vector.tensor_reduce(
                    asrc[:, c * CH:(c + 1) * CH], u3, axis=AxX, op=Alu.add)

            # ---- P = cnt*exp(leaky_relu(a_src+a_dst)); M; y-agg (halves) ----
            ynT = ps2.tile([128, 4 * T], f32, tag="ynT")
            M = wk.tile([128, 4 * T], bf16, tag="M")
            TH = T // 2
            for h in range(2):
                hs = slice(h * TH, (h + 1) * TH)
                z = wk.tile([128, TH], f32, tag="z")
                nc.vector.tensor_tensor(z[:], asrc[:, hs], adbc[:, hs],
                                        op=Alu.add)
                e = wk.tile([128, TH], f32, tag="e")
                nc.vector.scalar_tensor_tensor(
                    e[:], z[:], 0.2, z[:], op0=Alu.mult, op1=Alu.max)
                ex = wk.tile([128, TH], f32, tag="ex")
                nc.scalar.activation(ex[:], e[:], Act.Exp)
                P = wk.tile([128, TH], f32, tag="P")
                nc.vector.tensor_tensor(P[:], ex[:], cnt[:, hs], op=Alu.mult)
                Mv = M[:].rearrange("p (i j) -> p i j", j=4)
                for j in range(4):
                    nc.vector.tensor_scalar(
                        Mv[:, hs, j], P[:], Bm[:, j:j + 1], None, op0=Alu.mult)
                for i in range(h * TH, (h + 1) * TH):
                    nc.tensor.matmul(
                        ynT[0:SA, 4 * i:4 * (i + 1)],
                        x[:, SA * i:SA * (i + 1)],
                        M[:, 4 * i:4 * (i + 1)],
                        start=True, stop=True)

            # ---- normalize by denominator (row 64 of y^T) ----
            ysb = wk.tile([S + 1, GPC], f32, tag="ysb")
            nc.scalar.copy(ysb[:], ynT[0:S + 1, :])
            dn = wk.tile([1, GPC], f32, tag="dn")
            nc.vector.tensor_scalar(
                dn[:], ysb[S:S + 1, :], eps[:], None, op0=Alu.add)
            rp = wk.tile([1, GPC], f32, tag="rp")
            nc.vector.reciprocal_approx_fast(rp[:], dn[:])
            rbc = ps1.tile([S, GPC], f32, tag="rbc")
            nc.tensor.matmul(rbc[:], ones64[:], rp[:], start=True, stop=True)
            ynrm = wk.tile([S, GPC], bf16, tag="ynrm")
            nc.vector.tensor_tensor(ynrm[:], ysb[0:S, :], rbc[:], op=Alu.mult)

            # ---- project + bias + sigmoid ----
            hT = ps1.tile([H, GPC], f32, tag="hT")
            nc.tensor.matmul(hT[:], Wb[:], ynrm[:], start=True, stop=True)
            sg = wk.tile([H, GPC], bf16, tag="sig")
            nc.scalar.activation(sg[:], hT[:], Act.Sigmoid, bias=bias[:])
            sig[p] = sg

        # ---- combine branches + MLP head ----
        prod = wk.tile([H, GPC], bf16, tag="prod")
        nc.vector.tensor_tensor(prod[:], sig["u"][:], sig["d"][:], op=Alu.mult)
        o_ps = ps1.tile([1, GPC], f32, tag="o_ps")
        nc.tensor.matmul(o_ps[:], mlpW[:], prod[:], start=True, stop=True)
        o_sb = wk.tile([1, GPC], f32, tag="o_sb")
        nc.vector.tensor_scalar(
            o_sb[:], o_ps[:], mlpb[:], None, op0=Alu.add)
        nc.sync.dma_start(out_dram.ap(), o_sb[:])

    nc.compile()
    return nc


def _get_module():
    if "nc" not in _CACHE:
        _CACHE["nc"] = _build_module()
    return _CACHE["nc"]


def _prep_branch(x, ei, W, att_src, att_dst, bias):
    """Host-side sharding + graph-format prep for one branch."""
    import ml_dtypes
    bf = ml_dtypes.bfloat16
    x = np.asarray(x, np.float32)
    src = np.asarray(ei[0]).astype(np.int64)
    dst = np.asarray(ei[1]).astype(np.int64)
    W = np.asarray(W, np.float32)
    w_src = (W @ np.asarray(att_src, np.float32)).astype(np.float32)
    w_dst = (W @ np.asarray(att_dst, np.float32)).astype(np.float32)

    valid = (dst % NPG) == (NPG - 1)
    cnt = np.bincount(src[valid], minlength=N).astype(np.float32)

    per_core = []
    for c in range(NC):
        xs = x[c * NPC:(c + 1) * NPC]
        xab = np.zeros((T, 128, SA), np.float32)
        xab[:, :, :S] = xs.reshape(T, 128, S)
        xab[:, :, S] = 1.0
        xab = np.ascontiguousarray(
            xab.transpose(1, 0, 2).reshape(128, T * SA)).astype(bf)
        cnt_t = np.ascontiguousarray(
            cnt[c * NPC:(c + 1) * NPC].reshape(T, 128).T)
        xlast = np.ascontiguousarray(
            xs[NPG - 1::NPG].reshape(128, 4 * S)).astype(bf)
        per_core.append({"xab": xab, "cnt": cnt_t, "xlast": xlast})

    wsrc_rep = np.zeros((128, CH * SA), np.float32)
    wsrc_rep.reshape(128, CH, SA)[:, :, :S] = w_src
    wdst_rep = np.broadcast_to(w_dst, (128, 4, S)).reshape(128, 4 * S)
    shared = {
        "wsrc": wsrc_rep.astype(bf),
        "wdst": wdst_rep.astype(bf),
        "W": W.astype(bf),
        "bias": np.asarray(bias, np.float32).reshape(H, 1),
    }
    return per_core, shared


def _build_in_maps(inputs):
    import ml_dtypes
    bf = ml_dtypes.bfloat16
    pcs = {}
    shareds = {}
    pcs["u"], shareds["u"] = _prep_branch(
        inputs["up_x"], inputs["up_edge_index"], inputs["up_W"],
        inputs["up_att_src"], inputs["up_att_dst"], inputs["up_bias"])
    pcs["d"], shareds["d"] = _prep_branch(
        inputs["down_x"], inputs["down_edge_index"], inputs["down_W"],
        inputs["down_att_src"], inputs["down_att_dst"], inputs["down_bias"])

    pp = np.arange(128)
    Qm = np.zeros((4, 128), np.float32)
    Qm[pp // 32, pp] = 1.0
    Bm = np.zeros((128, 4), np.float32)
    Bm[pp, pp // 32] = 1.0

    common = {
        "Qm": Qm.astype(bf),
        "Bm": Bm,
        "ones64": np.ones((1, S), np.float32),
        "ident": np.eye(128, dtype=np.float32),
        "mlpW": np.asarray(inputs["mlp_W"], np.float32).reshape(H, 1).astype(bf),
        "mlpb": np.asarray(inputs["mlp_b"], np.float32).reshape(1, 1),
        "eps": np.full((1, 1), 1e-16, np.float32),
    }
    for p in ("u", "d"):
        for k, v in shareds[p].items():
            common[f"{p}_{k}"] = v

    in_maps = []
    for c in range(NC):
        m = dict(common)
        for p in ("u", "d"):
            for k, v in pcs[p][c].items():
                m[f"{p}_{k}"] = v
        in_maps.append(m)
    return in_maps


def kernel(**inputs):
    from concourse.bass_utils import run_bass_kernel_spmd

    nc = _get_module()
    in_maps = _build_in_maps(inputs)
    res = run_bass_kernel_spmd(nc, in_maps, core_ids=list(range(NC)))
    out = np.concatenate(
        [np.asarray(r["out"], np.float32).reshape(GPC) for r in res.results])
    return out.reshape(G, 1)


# revision 11
# speedup vs baseline: 1.1042x; 1.1042x over previous
"""Trainium2 Bass kernel for nn_Event_Critic_Net (dual-branch GAT critic).

Math: the reference only reads the GAT output at the LAST node of each
graph (graphs are 32 contiguous nodes), so only edges whose dst is a
graph's last node contribute.  For those edges the softmax-weighted
aggregation commutes with the linear projection W:

    out_g = sigmoid( (sum_n alpha[n] * x[n,:]) @ W + bias )
    alpha[n] = cnt[n]*exp(e[n]) / (sum_n cnt[n]*exp(e[n]) + 1e-16)
    e[n] = leaky_relu(x[n]. w_src + x[last(g)]. w_dst),  w_* = W @ att_*

cnt[n] = number of edges (n -> last(g(n))).  Graph-structure prep
(edge counts, tiling, transposed copy, weight replication) happens on
host; all FLOPs on device.  Sharding: graphs are data-parallel across
the 8 cores (core c owns graphs [c*512,(c+1)*512)).

x is shipped twice in bf16: node-major (y aggregation, PE contracts
over nodes) and s-major `xt` (attention logits, PE contracts over
features).  PSUM accumulates fp32; softmax scalars stay fp32.
"""

import numpy as np
from contextlib import ExitStack

NC = 8            # cores
N = 131072        # nodes total
G = 4096          # graphs
NPG = 32          # nodes per graph
S = 64            # state size
H = 128           # hidden size
NPC = N // NC     # 16384 nodes per core
GPC = G // NC     # 512 graphs per core
T = NPC // 128    # 128 node-tiles per core
SA = 66           # x columns: 64 features | ones@64 | zero pad
TH = T // 2       # half-branch tiles

_CACHE = {}


def _build_module():
    import concourse.tile as tile
    from concourse import bacc, mybir
    from concourse.alu_op_type import AluOpType as Alu

    f32 = mybir.dt.float32
    bf16 = mybir.dt.bfloat16
    Act = mybir.ActivationFunctionType
    AxX = mybir.AxisListType.X

    nc = bacc.Bacc("TRN2", target_bir_lowering=False, debug=False,
                   num_devices=NC)

    dram = {}

    def din(name, shape, dt=f32):
        dram[name] = nc.dram_tensor(name, shape, dt, kind="ExternalInput")

    for p in ("u", "d"):
        din(f"{p}_xab", [128, T * SA], bf16)
        din(f"{p}_xt", [128, NPC // 2], bf16)
        din(f"{p}_cnt", [128, T])
        din(f"{p}_xlast", [128, 4 * S], bf16)
        din(f"{p}_wv2", [128, 2], bf16)
        din(f"{p}_wdst", [128, 4 * S], bf16)
        din(f"{p}_bias", [128, 1])
        din(f"{p}_W", [S, H], bf16)
    din("Qm", [4, 128], bf16)
    din("Bm", [128, 4])
    din("ones64", [1, S])
    din("ident", [128, 128])
    din("mlpW", [H, 1], bf16)
    din("mlpb", [1, 1])
    din("eps", [1, 1])
    out_dram = nc.dram_tensor("out", [1, GPC], f32, kind="ExternalOutput")

    with tile.TileContext(nc) as tc, ExitStack() as ctx:
        const = ctx.enter_context(tc.tile_pool(name="const", bufs=1))
        xp = ctx.enter_context(tc.tile_pool(name="xp", bufs=2))
        wk = ctx.enter_context(tc.tile_pool(name="wk", bufs=2))
        ps1 = ctx.enter_context(tc.tile_pool(name="ps1", bufs=1, space="PSUM"))
        ps2 = ctx.enter_context(tc.tile_pool(name="ps2", bufs=2, space="PSUM"))

        def cload(name, shape, dt=f32):
            t = const.tile(shape, dt, tag=name)
            nc.sync.dma_start(t[:], dram[name].ap())
            return t

        Qm = cload("Qm", [4, 128], bf16)
        Bm = cload("Bm", [128, 4])
        ones64 = cload("ones64", [1, S])
        ident = cload("ident", [128, 128])
        mlpW = cload("mlpW", [H, 1], bf16)
        mlpb = cload("mlpb", [1, 1])
        eps = cload("eps", [1, 1])

        sig = {}
        for p in ("u", "d"):
            wv2 = cload(f"{p}_wv2", [128, 2], bf16)
            wdst = cload(f"{p}_wdst", [128, 4 * S], bf16)
            Wb = cload(f"{p}_W", [S, H], bf16)
            bias = cload(f"{p}_bias", [128, 1])

            # ---- big loads, chunked for DMA/compute overlap ----
            x = xp.tile([128, T * SA], bf16, tag="x")
            for c in range(8):
                sl = slice(c * (T // 8) * SA, (c + 1) * (T // 8) * SA)
                nc.sync.dma_start(x[:, sl], dram[f"{p}_xab"].ap()[:, sl])
            xt = xp.tile([128, NPC // 2], bf16, tag="xt")
            for c in range(4):
                sl = slice(c * NPC // 8, (c + 1) * NPC // 8)
                nc.sync.dma_start(xt[:, sl], dram[f"{p}_xt"].ap()[:, sl])
            cnt = wk.tile([128, T], f32, tag="cnt")
            nc.sync.dma_start(cnt[:], dram[f"{p}_cnt"].ap())
            xl = wk.tile([128, 4 * S], bf16, tag="xl")
            nc.sync.dma_start(xl[:], dram[f"{p}_xlast"].ap())

            # ---- a_dst at last nodes: mult+reduce, transpose, broadcast ----
            tmp4 = wk.tile([128, 4 * S], bf16, tag="tmp4")
            nc.vector.tensor_tensor(tmp4[:], xl[:], wdst[:], op=Alu.mult)
            adst = wk.tile([128, 4], f32, tag="adst")
            nc.vector.tensor_reduce(
                adst[:], tmp4[:].rearrange("p (j s) -> p j s", s=S),
                axis=AxX, op=Alu.add)
            tp = ps1.tile([4, 128], f32, tag="tp")
            nc.tensor.transpose(tp[:], adst[:], ident[:])
            adT = wk.tile([4, 128], bf16, tag="adT")
            nc.vector.tensor_copy(adT[:], tp[:])
            adbc = ps1.tile([128, T], f32, tag="adbc")
            nc.tensor.matmul(adbc[:], Qm[:], adT[:], start=True, stop=True)

            # ---- a_src (and a_dst) per node on PE: xt^T @ [w_src|w_dst] ----
            # node-tile i (nodes 128i..128i+128): half k=i//64, col c=i%64
            asps = ps1.tile([128, 2 * T], f32, tag="asps")
            for i in range(T):
                k, c = divmod(i, 64)
                nc.tensor.matmul(
                    asps[0:128, 2 * i:2 * i + 2],
                    xt[64 * k:64 * k + 64, 128 * c:128 * c + 128],
                    wv2[64 * k:64 * k + 64, :],
                    start=True, stop=True)

            # ---- P = cnt*exp(leaky_relu(a_src+a_dst)); M (halves) ----
            M = wk.tile([128, 4 * T], bf16, tag="M")
            Mv = M[:].rearrange("p (i j) -> p i j", j=4)
            for h in range(2):
                hs = slice(h * TH, (h + 1) * TH)
                asrc = wk.tile([128, TH], f32, tag="asrc")
                nc.vector.tensor_copy(
                    asrc[:], asps[:, 2 * h * TH:2 * (h + 1) * TH:2])
                z = wk.tile([128, TH], f32, tag="z")
                nc.vector.tensor_tensor(z[:], asrc[:], adbc[:, hs],
                                        op=Alu.add)
                e = wk.tile([128, TH], f32, tag="e")
                nc.vector.scalar_tensor_tensor(
                    e[:], z[:], 0.2, z[:], op0=Alu.mult, op1=Alu.max)
                ex = wk.tile([128, TH], f32, tag="ex")
                nc.scalar.activation(ex[:], e[:], Act.Exp)
                P = wk.tile([128, TH], f32, tag="P")
                nc.vector.tensor_tensor(P[:], ex[:], cnt[:, hs], op=Alu.mult)
                for j in range(4):
                    nc.vector.tensor_scalar(
                        Mv[:, hs, j], P[:], Bm[:, j:j + 1], None, op0=Alu.mult)

            # ---- y^T aggregation: 128 small matmuls ----
            ynT = ps2.tile([128, 4 * T], f32, tag="ynT")
            for i in range(T):
                nc.tensor.matmul(
                    ynT[0:SA, 4 * i:4 * (i + 1)],
                    x[:, SA * i:SA * (i + 1)],
                    M[:, 4 * i:4 * (i + 1)],
                    start=True, stop=True)

            # ---- normalize by denominator (row 64 of y^T) ----
            ysb = wk.tile([S + 1, GPC], f32, tag="ysb")
            nc.scalar.copy(ysb[:], ynT[0:S + 1, :])
            dn = wk.tile([1, GPC], f32, tag="dn")
            nc.vector.tensor_scalar(
                dn[:], ysb[S:S + 1, :], eps[:], None, op0=Alu.add)
            rp = wk.tile([1, GPC], f32, tag="rp")
            nc.vector.reciprocal_approx_fast(rp[:], dn[:])
            rbc = ps1.tile([S, GPC], f32, tag="rbc")
            nc.tensor.matmul(rbc[:], ones64[:], rp[:], start=True, stop=True)
            ynrm = wk.tile([S, GPC], bf16, tag="ynrm")
            nc.vector.tensor_tensor(ynrm[:], ysb[0:S, :], rbc[:], op=Alu.mult)

            # ---- project + bias + sigmoid ----
            hT = ps1.tile([H, GPC], f32, tag="hT")
            nc.tensor.matmul(hT[:], Wb[:], ynrm[:], start=True, stop=True)
            sg = wk.tile([H, GPC], bf16, tag="sig")
            nc.scalar.activation(sg[:], hT[:], Act.Sigmoid, bias=bias[:])
            sig[p] = sg

        # ---- combine branches + MLP head ----
        prod = wk.tile([H, GPC], bf16, tag="prod")
        nc.vector.tensor_tensor(prod[:], sig["u"][:], sig["d"][:], op=Alu.mult)
        o_ps = ps1.tile([1, GPC], f32, tag="o_ps")
        nc.tensor.matmul(o_ps[:], mlpW[:], prod[:], start=True, stop=True)
        o_sb = wk.tile([1, GPC], f32, tag="o_sb")
        nc.vector.tensor_scalar(
            o_sb[:], o_ps[:], mlpb[:], None, op0=Alu.add)
        nc.sync.dma_start(out_dram.ap(), o_sb[:])

    nc.compile()
    return nc


def _get_module():
    if "nc" not in _CACHE:
        _CACHE["nc"] = _build_module()
    return _CACHE["nc"]


def _prep_branch(x, ei, W, att_src, att_dst, bias):
    """Host-side sharding + graph-format prep for one branch."""
    import ml_dtypes
    bf = ml_dtypes.bfloat16
    x = np.asarray(x, np.float32)
    src = np.asarray(ei[0]).astype(np.int64)
    dst = np.asarray(ei[1]).astype(np.int64)
    W = np.asarray(W, np.float32)
    w_src = (W @ np.asarray(att_src, np.float32)).astype(np.float32)
    w_dst = (W @ np.asarray(att_dst, np.float32)).astype(np.float32)

    valid = (dst % NPG) == (NPG - 1)
    cnt = np.bincount(src[valid], minlength=N).astype(np.float32)

    per_core = []
    for c in range(NC):
        xs = x[c * NPC:(c + 1) * NPC]
        xab = np.zeros((T, 128, SA), np.float32)
        xab[:, :, :S] = xs.reshape(T, 128, S)
        xab[:, :, S] = 1.0
        xab = np.ascontiguousarray(
            xab.transpose(1, 0, 2).reshape(128, T * SA)).astype(bf)
        # xt[64k+s, m] = x[8192k + m, s]
        xtv = xs.reshape(2, NPC // 2, S).transpose(0, 2, 1)
        xtv = np.ascontiguousarray(xtv.reshape(128, NPC // 2)).astype(bf)
        cnt_t = np.ascontiguousarray(
            cnt[c * NPC:(c + 1) * NPC].reshape(T, 128).T)
        xlast = np.ascontiguousarray(
            xs[NPG - 1::NPG].reshape(128, 4 * S)).astype(bf)
        per_core.append({"xab": xab, "xt": xtv, "cnt": cnt_t, "xlast": xlast})

    wv2 = np.stack([w_src, w_dst], axis=1)          # [64, 2]
    wv2 = np.concatenate([wv2, wv2], axis=0)        # [128, 2] both halves
    wdst_rep = np.broadcast_to(w_dst, (128, 4, S)).reshape(128, 4 * S)
    shared = {
        "wv2": wv2.astype(bf),
        "wdst": wdst_rep.astype(bf),
        "W": W.astype(bf),
        "bias": np.asarray(bias, np.float32).reshape(H, 1),
    }
    return per_core, shared


def _build_in_maps(inputs):
    import ml_dtypes
    bf = ml_dtypes.bfloat16
    pcs = {}
    shareds = {}
    pcs["u"], shareds["u"] = _prep_branch(
        inputs["up_x"], inputs["up_edge_index"], inputs["up_W"],
        inputs["up_att_src"], inputs["up_att_dst"], inputs["up_bias"])
    pcs["d"], shareds["d"] = _prep_branch(
        inputs["down_x"], inputs["down_edge_index"], inputs["down_W"],
        inputs["down_att_src"], inputs["down_att_dst"], inputs["down_bias"])

    pp = np.arange(128)
    Qm = np.zeros((4, 128), np.float32)
    Qm[pp // 32, pp] = 1.0
    Bm = np.zeros((128, 4), np.float32)
    Bm[pp, pp // 32] = 1.0

    common = {
        "Qm": Qm.astype(bf),
        "Bm": Bm,
        "ones64": np.ones((1, S), np.float32),
        "ident": np.eye(128, dtype=np.float32),
        "mlpW": np.asarray(inputs["mlp_W"], np.float32).reshape(H, 1).astype(bf),
        "mlpb": np.asarray(inputs["mlp_b"], np.float32).reshape(1, 1),
        "eps": np.full((1, 1), 1e-16, np.float32),
    }
    for p in ("u", "d"):
        for k, v in shareds[p].items():
            common[f"{p}_{k}"] = v

    in_maps = []
    for c in range(NC):
        m = dict(common)
        for p in ("u", "d"):
            for k, v in pcs[p][c].items():
                m[f"{p}_{k}"] = v
        in_maps.append(m)
    return in_maps


def kernel(**inputs):
    from concourse.bass_utils import run_bass_kernel_spmd

    nc = _get_module()
    in_maps = _build_in_maps(inputs)
    res = run_bass_kernel_spmd(nc, in_maps, core_ids=list(range(NC)))
    out = np.concatenate(
        [np.asarray(r["out"], np.float32).reshape(GPC) for r in res.results])
    return out.reshape(G, 1)


# revision 13
# speedup vs baseline: 1.2209x; 1.1057x over previous
"""Trainium2 Bass kernel for nn_Event_Critic_Net (dual-branch GAT critic).

Math: the reference only reads the GAT output at the LAST node of each
graph (graphs are 32 contiguous nodes), so only edges whose dst is a
graph's last node contribute.  For those edges the softmax-weighted
aggregation commutes with the linear projection W:

    out_g = sigmoid( (sum_n alpha[n] * x[n,:]) @ W + bias )
    alpha[n] = cnt[n]*exp(e[n]) / (sum_n cnt[n]*exp(e[n]) + 1e-16)
    e[n] = leaky_relu(x[n]. w_src + x[last(g)]. w_dst),  w_* = W @ att_*

cnt[n] = number of edges (n -> last(g(n))).  Graph-structure prep
(edge counts, tiling, transposed copy, weight replication) happens on
host; all FLOPs on device.  Sharding: graphs are data-parallel across
the 8 cores (core c owns graphs [c*512,(c+1)*512)).

x is shipped twice in bf16: node-major (y aggregation, PE contracts
over nodes) and s-major `xt` (attention logits, PE contracts over
features).  PSUM accumulates fp32; softmax scalars stay fp32.
"""

import numpy as np
from contextlib import ExitStack

NC = 8            # cores
N = 131072        # nodes total
G = 4096          # graphs
NPG = 32          # nodes per graph
S = 64            # state size
H = 128           # hidden size
NPC = N // NC     # 16384 nodes per core
GPC = G // NC     # 512 graphs per core
T = NPC // 128    # 128 node-tiles per core
SA = 66           # x columns: 64 features | ones@64 | zero pad
TH = T // 2       # half-branch tiles

_CACHE = {}


def _build_module():
    import concourse.tile as tile
    from concourse import bacc, mybir
    from concourse.alu_op_type import AluOpType as Alu

    f32 = mybir.dt.float32
    bf16 = mybir.dt.bfloat16
    Act = mybir.ActivationFunctionType
    AxX = mybir.AxisListType.X

    nc = bacc.Bacc("TRN2", target_bir_lowering=False, debug=False,
                   num_devices=NC)

    dram = {}

    def din(name, shape, dt=f32):
        dram[name] = nc.dram_tensor(name, shape, dt, kind="ExternalInput")

    for p in ("u", "d"):
        din(f"{p}_xab", [128, T * SA], bf16)
        din(f"{p}_xt", [128, NPC // 2], bf16)
        din(f"{p}_cnt", [128, T])
        din(f"{p}_xlast", [128, 4 * S], bf16)
        din(f"{p}_wv4", [128, 4], bf16)
        din(f"{p}_wdst", [128, 4 * S], bf16)
        din(f"{p}_bias", [128, 1])
        din(f"{p}_W", [S, H], bf16)
    din("Qm", [4, 128], bf16)
    din("Bm", [128, 4])
    din("ones64", [1, S])
    din("ident", [128, 128])
    din("mlpW", [H, 1], bf16)
    din("mlpb", [1, 1])
    din("eps", [1, 1])
    out_dram = nc.dram_tensor("out", [1, GPC], f32, kind="ExternalOutput")

    with tile.TileContext(nc) as tc, ExitStack() as ctx:
        const = ctx.enter_context(tc.tile_pool(name="const", bufs=1))
        xp = ctx.enter_context(tc.tile_pool(name="xp", bufs=2))
        wk = ctx.enter_context(tc.tile_pool(name="wk", bufs=2))
        ps1 = ctx.enter_context(tc.tile_pool(name="ps1", bufs=1, space="PSUM"))
        ps2 = ctx.enter_context(tc.tile_pool(name="ps2", bufs=2, space="PSUM"))

        def cload(name, shape, dt=f32):
            t = const.tile(shape, dt, tag=name)
            nc.sync.dma_start(t[:], dram[name].ap())
            return t

        Qm = cload("Qm", [4, 128], bf16)
        Bm = cload("Bm", [128, 4])
        ones64 = cload("ones64", [1, S])
        ident = cload("ident", [128, 128])
        mlpW = cload("mlpW", [H, 1], bf16)
        mlpb = cload("mlpb", [1, 1])
        eps = cload("eps", [1, 1])

        sig = {}
        st = {}
        # ---- phase A (both branches): loads + per-node attention logits ----
        for p in ("u", "d"):
            s = st[p] = {}
            wv4 = cload(f"{p}_wv4", [128, 4], bf16)
            wdst = cload(f"{p}_wdst", [128, 4 * S], bf16)
            s["Wb"] = cload(f"{p}_W", [S, H], bf16)
            s["bias"] = cload(f"{p}_bias", [128, 1])

            x = xp.tile([128, T * SA], bf16, tag="x", name=f"x_{p}")
            s["x"] = x
            for c in range(8):
                sl = slice(c * (T // 8) * SA, (c + 1) * (T // 8) * SA)
                nc.sync.dma_start(x[:, sl], dram[f"{p}_xab"].ap()[:, sl])
            xt = xp.tile([128, NPC // 2], bf16, tag="xt")
            for c in range(4):
                sl = slice(c * NPC // 8, (c + 1) * NPC // 8)
                nc.sync.dma_start(xt[:, sl], dram[f"{p}_xt"].ap()[:, sl])
            cnt = wk.tile([128, T], f32, tag="cnt", name=f"cnt_{p}")
            s["cnt"] = cnt
            nc.sync.dma_start(cnt[:], dram[f"{p}_cnt"].ap())
            xl = wk.tile([128, 4 * S], bf16, tag="xl")
            nc.sync.dma_start(xl[:], dram[f"{p}_xlast"].ap())

            # a_dst at last nodes: mult+reduce, transpose, broadcast
            tmp4 = wk.tile([128, 4 * S], bf16, tag="tmp4")
            nc.vector.tensor_tensor(tmp4[:], xl[:], wdst[:], op=Alu.mult)
            adst = wk.tile([128, 4], f32, tag="adst")
            nc.vector.tensor_reduce(
                adst[:], tmp4[:].rearrange("p (j s) -> p j s", s=S),
                axis=AxX, op=Alu.add)
            tp = ps1.tile([4, 128], f32, tag="mix")
            nc.tensor.transpose(tp[:], adst[:], ident[:])
            adT = wk.tile([4, 128], bf16, tag="adT")
            nc.vector.tensor_copy(adT[:], tp[:])
            adbc_ps = ps1.tile([128, T], f32, tag="adbc")
            nc.tensor.matmul(adbc_ps[:], Qm[:], adT[:], start=True, stop=True)
            adbc = wk.tile([128, T], f32, tag="adbcs", name=f"adbcs_{p}")
            s["adbc"] = adbc
            nc.vector.tensor_copy(adbc[:], adbc_ps[:])

            # a_src per node on PE: one f=4 matmul covers two node-tiles
            # (chunk c: cols 4c+0/1 = tile c, cols 4c+2/3 = tile 64+c)
            asps = ps2.tile([128, 2 * T], f32, tag="asps", name=f"asps_{p}")
            s["asps"] = asps
            for c in range(T // 2):
                nc.tensor.matmul(
                    asps[0:128, 4 * c:4 * c + 4],
                    xt[:, 128 * c:128 * c + 128],
                    wv4[:],
                    start=True, stop=True)

        # ---- phase B (both branches): P/M, aggregation, normalize ----
        for p in ("u", "d"):
            s = st[p]
            x, cnt, adbc, asps = s["x"], s["cnt"], s["adbc"], s["asps"]
            M = wk.tile([128, 4 * T], bf16, tag="M")
            Mv = M[:].rearrange("p (i j) -> p i j", j=4)
            for h in range(2):
                hs = slice(h * TH, (h + 1) * TH)
                asrc = wk.tile([128, TH], f32, tag="asrc")
                nc.vector.tensor_copy(asrc[:], asps[:, 2 * h::4])
                z = wk.tile([128, TH], f32, tag="z")
                nc.vector.tensor_tensor(z[:], asrc[:], adbc[:, hs],
                                        op=Alu.add)
                e = wk.tile([128, TH], f32, tag="e")
                nc.vector.scalar_tensor_tensor(
                    e[:], z[:], 0.2, z[:], op0=Alu.mult, op1=Alu.max)
                ex = wk.tile([128, TH], f32, tag="ex")
                nc.scalar.activation(ex[:], e[:], Act.Exp)
                P = wk.tile([128, TH], f32, tag="P")
                nc.vector.tensor_tensor(P[:], ex[:], cnt[:, hs], op=Alu.mult)
                for j in range(4):
                    nc.vector.tensor_scalar(
                        Mv[:, hs, j], P[:], Bm[:, j:j + 1], None, op0=Alu.mult)

            ynT = ps2.tile([128, 4 * T], f32, tag="ynT")
            for i in range(T):
                nc.tensor.matmul(
                    ynT[0:SA, 4 * i:4 * (i + 1)],
                    x[:, SA * i:SA * (i + 1)],
                    M[:, 4 * i:4 * (i + 1)],
                    start=True, stop=True)

            # normalize by denominator (row 64 of y^T)
            ysb = wk.tile([S + 1, GPC], f32, tag="ysb")
            nc.scalar.copy(ysb[:], ynT[0:S + 1, :])
            dn = wk.tile([1, GPC], f32, tag="dn")
            nc.vector.tensor_scalar(
                dn[:], ysb[S:S + 1, :], eps[:], None, op0=Alu.add)
            rp = wk.tile([1, GPC], f32, tag="rp")
            nc.vector.reciprocal_approx_fast(rp[:], dn[:])
            rbc = ps1.tile([S, GPC], f32, tag="mix")
            nc.tensor.matmul(rbc[:], ones64[:], rp[:], start=True, stop=True)
            ynrm = wk.tile([S, GPC], bf16, tag="ynrm")
            nc.vector.tensor_tensor(ynrm[:], ysb[0:S, :], rbc[:], op=Alu.mult)

            # project + bias + sigmoid
            hT = ps1.tile([H, GPC], f32, tag="hT")
            nc.tensor.matmul(hT[:], s["Wb"][:], ynrm[:], start=True, stop=True)
            sg = wk.tile([H, GPC], bf16, tag="sig")
            nc.scalar.activation(sg[:], hT[:], Act.Sigmoid, bias=s["bias"][:])
            sig[p] = sg

        # ---- combine branches + MLP head ----
        prod = wk.tile([H, GPC], bf16, tag="prod")
        nc.vector.tensor_tensor(prod[:], sig["u"][:], sig["d"][:], op=Alu.mult)
        o_ps = ps1.tile([1, GPC], f32, tag="mix")
        nc.tensor.matmul(o_ps[:], mlpW[:], prod[:], start=True, stop=True)
        o_sb = wk.tile([1, GPC], f32, tag="o_sb")
        nc.vector.tensor_scalar(
            o_sb[:], o_ps[:], mlpb[:], None, op0=Alu.add)
        nc.sync.dma_start(out_dram.ap(), o_sb[:])

    nc.compile()
    return nc


def _get_module():
    if "nc" not in _CACHE:
        _CACHE["nc"] = _build_module()
    return _CACHE["nc"]


def _prep_branch(x, ei, W, att_src, att_dst, bias):
    """Host-side sharding + graph-format prep for one branch."""
    import ml_dtypes
    bf = ml_dtypes.bfloat16
    x = np.asarray(x, np.float32)
    src = np.asarray(ei[0]).astype(np.int64)
    dst = np.asarray(ei[1]).astype(np.int64)
    W = np.asarray(W, np.float32)
    w_src = (W @ np.asarray(att_src, np.float32)).astype(np.float32)
    w_dst = (W @ np.asarray(att_dst, np.float32)).astype(np.float32)

    valid = (dst % NPG) == (NPG - 1)
    cnt = np.bincount(src[valid], minlength=N).astype(np.float32)

    per_core = []
    for c in range(NC):
        xs = x[c * NPC:(c + 1) * NPC]
        xab = np.zeros((T, 128, SA), np.float32)
        xab[:, :, :S] = xs.reshape(T, 128, S)
        xab[:, :, S] = 1.0
        xab = np.ascontiguousarray(
            xab.transpose(1, 0, 2).reshape(128, T * SA)).astype(bf)
        # xt[64k+s, m] = x[8192k + m, s]
        xtv = xs.reshape(2, NPC // 2, S).transpose(0, 2, 1)
        xtv = np.ascontiguousarray(xtv.reshape(128, NPC // 2)).astype(bf)
        cnt_t = np.ascontiguousarray(
            cnt[c * NPC:(c + 1) * NPC].reshape(T, 128).T)
        xlast = np.ascontiguousarray(
            xs[NPG - 1::NPG].reshape(128, 4 * S)).astype(bf)
        per_core.append({"xab": xab, "xt": xtv, "cnt": cnt_t, "xlast": xlast})

    wv4 = np.zeros((128, 4), np.float32)
    wv4[:S, 0] = w_src
    wv4[:S, 1] = w_dst
    wv4[S:, 2] = w_src
    wv4[S:, 3] = w_dst
    wdst_rep = np.broadcast_to(w_dst, (128, 4, S)).reshape(128, 4 * S)
    shared = {
        "wv4": wv4.astype(bf),
        "wdst": wdst_rep.astype(bf),
        "W": W.astype(bf),
        "bias": np.asarray(bias, np.float32).reshape(H, 1),
    }
    return per_core, shared


def _build_in_maps(inputs):
    import ml_dtypes
    bf = ml_dtypes.bfloat16
    pcs = {}
    shareds = {}
    pcs["u"], shareds["u"] = _prep_branch(
        inputs["up_x"], inputs["up_edge_index"], inputs["up_W"],
        inputs["up_att_src"], inputs["up_att_dst"], inputs["up_bias"])
    pcs["d"], shareds["d"] = _prep_branch(
        inputs["down_x"], inputs["down_edge_index"], inputs["down_W"],
        inputs["down_att_src"], inputs["down_att_dst"], inputs["down_bias"])

    pp = np.arange(128)
    Qm = np.zeros((4, 128), np.float32)
    Qm[pp // 32, pp] = 1.0
    Bm = np.zeros((128, 4), np.float32)
    Bm[pp, pp // 32] = 1.0

    common = {
        "Qm": Qm.astype(bf),
        "Bm": Bm,
        "ones64": np.ones((1, S), np.float32),
        "ident": np.eye(128, dtype=np.float32),
        "mlpW": np.asarray(inputs["mlp_W"], np.float32).reshape(H, 1).astype(bf),
        "mlpb": np.asarray(inputs["mlp_b"], np.float32).reshape(1, 1),
        "eps": np.full((1, 1), 1e-16, np.float32),
    }
    for p in ("u", "d"):
        for k, v in shareds[p].items():
            common[f"{p}_{k}"] = v

    in_maps = []
    for c in range(NC):
        m = dict(common)
        for p in ("u", "d"):
            for k, v in pcs[p][c].items():
                m[f"{p}_{k}"] = v
        in_maps.append(m)
    return in_maps


def kernel(**inputs):
    from concourse.bass_utils import run_bass_kernel_spmd

    nc = _get_module()
    in_maps = _build_in_maps(inputs)
    res = run_bass_kernel_spmd(nc, in_maps, core_ids=list(range(NC)))
    out = np.concatenate(
        [np.asarray(r["out"], np.float32).reshape(GPC) for r in res.results])
    return out.reshape(G, 1)


# revision 14
# speedup vs baseline: 1.2695x; 1.0398x over previous
"""Trainium2 Bass kernel for nn_Event_Critic_Net (dual-branch GAT critic).

Math: the reference only reads the GAT output at the LAST node of each
graph (graphs are 32 contiguous nodes), so only edges whose dst is a
graph's last node contribute.  For those edges the softmax-weighted
aggregation commutes with the linear projection W:

    out_g = sigmoid( (sum_n alpha[n] * x[n,:]) @ W + bias )
    alpha[n] = cnt[n]*exp(e[n]) / (sum_n cnt[n]*exp(e[n]) + 1e-16)
    e[n] = leaky_relu(x[n]. w_src + x[last(g)]. w_dst),  w_* = W @ att_*

cnt[n] = number of edges (n -> last(g(n))).  Graph-structure prep
(edge counts, tiling, transposed copy, weight replication) happens on
host; all FLOPs on device.  Sharding: graphs are data-parallel across
the 8 cores (core c owns graphs [c*512,(c+1)*512)).

x is shipped twice in bf16: node-major (y aggregation, PE contracts
over nodes) and s-major `xt` (attention logits, PE contracts over
features).  PSUM accumulates fp32; softmax scalars stay fp32.
"""

import numpy as np
from contextlib import ExitStack

NC = 8            # cores
N = 131072        # nodes total
G = 4096          # graphs
NPG = 32          # nodes per graph
S = 64            # state size
H = 128           # hidden size
NPC = N // NC     # 16384 nodes per core
GPC = G // NC     # 512 graphs per core
T = NPC // 128    # 128 node-tiles per core
SA = 66           # x columns: 64 features | ones@64 | zero pad
TH = T // 2       # half-branch tiles

_CACHE = {}


def _build_module():
    import concourse.tile as tile
    from concourse import bacc, mybir
    from concourse.alu_op_type import AluOpType as Alu

    f32 = mybir.dt.float32
    bf16 = mybir.dt.bfloat16
    Act = mybir.ActivationFunctionType
    AxX = mybir.AxisListType.X

    nc = bacc.Bacc("TRN2", target_bir_lowering=False, debug=False,
                   num_devices=NC)

    dram = {}

    def din(name, shape, dt=f32):
        dram[name] = nc.dram_tensor(name, shape, dt, kind="ExternalInput")

    for p in ("u", "d"):
        din(f"{p}_xab", [128, T * SA], bf16)
        din(f"{p}_xt", [128, NPC // 2], bf16)
        din(f"{p}_cnt", [128, T])
        din(f"{p}_xlast", [128, 4 * S], bf16)
        din(f"{p}_wv4", [128, 4], bf16)
        din(f"{p}_wdst", [128, 4 * S], bf16)
        din(f"{p}_bias", [128, 1])
        din(f"{p}_W", [S, H], bf16)
    din("Qm", [4, 128], bf16)
    din("Bm", [128, 4])
    din("ones64", [1, S])
    din("ident", [128, 128])
    din("mlpW", [H, 1], bf16)
    din("mlpb", [1, 1])
    din("eps", [1, 1])
    out_dram = nc.dram_tensor("out", [1, GPC], f32, kind="ExternalOutput")

    with tile.TileContext(nc) as tc, ExitStack() as ctx:
        const = ctx.enter_context(tc.tile_pool(name="const", bufs=1))
        xp = ctx.enter_context(tc.tile_pool(name="xp", bufs=2))
        wk = ctx.enter_context(tc.tile_pool(name="wk", bufs=2))
        ps1 = ctx.enter_context(tc.tile_pool(name="ps1", bufs=1, space="PSUM"))
        ps2 = ctx.enter_context(tc.tile_pool(name="ps2", bufs=2, space="PSUM"))

        def cload(name, shape, dt=f32):
            t = const.tile(shape, dt, tag=name)
            nc.scalar.dma_start(t[:], dram[name].ap())
            return t

        Qm = cload("Qm", [4, 128], bf16)
        Bm = cload("Bm", [128, 4])
        ones64 = cload("ones64", [1, S])
        ident = cload("ident", [128, 128])
        mlpW = cload("mlpW", [H, 1], bf16)
        mlpb = cload("mlpb", [1, 1])
        eps = cload("eps", [1, 1])

        sig = {}
        st = {}
        # ---- phase A (both branches): loads + per-node attention logits ----
        for p in ("u", "d"):
            s = st[p] = {}
            wv4 = cload(f"{p}_wv4", [128, 4], bf16)
            wdst = cload(f"{p}_wdst", [128, 4 * S], bf16)
            s["Wb"] = cload(f"{p}_W", [S, H], bf16)
            s["bias"] = cload(f"{p}_bias", [128, 1])

            cnt = wk.tile([128, T], f32, tag="cnt", name=f"cnt_{p}")
            s["cnt"] = cnt
            nc.scalar.dma_start(cnt[:], dram[f"{p}_cnt"].ap())
            xl = wk.tile([128, 4 * S], bf16, tag="xl")
            nc.scalar.dma_start(xl[:], dram[f"{p}_xlast"].ap())
            xt = xp.tile([128, NPC // 2], bf16, tag="xt")
            for c in range(2):
                sl = slice(c * NPC // 4, (c + 1) * NPC // 4)
                nc.sync.dma_start(xt[:, sl], dram[f"{p}_xt"].ap()[:, sl])
            x = xp.tile([128, T * SA], bf16, tag="x", name=f"x_{p}")
            s["x"] = x
            nc.sync.dma_start(x[:], dram[f"{p}_xab"].ap())

            # a_dst at last nodes: mult+reduce, transpose, broadcast
            tmp4 = wk.tile([128, 4 * S], bf16, tag="tmp4")
            nc.vector.tensor_tensor(tmp4[:], xl[:], wdst[:], op=Alu.mult)
            adst = wk.tile([128, 4], f32, tag="adst")
            nc.vector.tensor_reduce(
                adst[:], tmp4[:].rearrange("p (j s) -> p j s", s=S),
                axis=AxX, op=Alu.add)
            tp = ps1.tile([4, 128], f32, tag="mix")
            nc.tensor.transpose(tp[:], adst[:], ident[:])
            adT = wk.tile([4, 128], bf16, tag="adT")
            nc.vector.tensor_copy(adT[:], tp[:])
            adbc_ps = ps1.tile([128, T], f32, tag="adbc")
            nc.tensor.matmul(adbc_ps[:], Qm[:], adT[:], start=True, stop=True)
            adbc = wk.tile([128, T], f32, tag="adbcs", name=f"adbcs_{p}")
            s["adbc"] = adbc
            nc.vector.tensor_copy(adbc[:], adbc_ps[:])

            # a_src per node on PE: one f=4 matmul covers two node-tiles
            # (chunk c: cols 4c+0/1 = tile c, cols 4c+2/3 = tile 64+c)
            asps = ps2.tile([128, 2 * T], f32, tag="asps", name=f"asps_{p}")
            s["asps"] = asps
            for c in range(T // 2):
                nc.tensor.matmul(
                    asps[0:128, 4 * c:4 * c + 4],
                    xt[:, 128 * c:128 * c + 128],
                    wv4[:],
                    start=True, stop=True)
            del xt

        # ---- phase B (both branches): P/M, aggregation, normalize ----
        for p in ("u", "d"):
            s = st[p]
            x, cnt, adbc, asps = s["x"], s["cnt"], s["adbc"], s["asps"]
            M = wk.tile([128, 4 * T], bf16, tag="M")
            Mv = M[:].rearrange("p (i j) -> p i j", j=4)
            for h in range(2):
                hs = slice(h * TH, (h + 1) * TH)
                asrc = wk.tile([128, TH], f32, tag="asrc")
                nc.vector.tensor_copy(asrc[:], asps[:, 2 * h::4])
                z = wk.tile([128, TH], f32, tag="z")
                nc.vector.tensor_tensor(z[:], asrc[:], adbc[:, hs],
                                        op=Alu.add)
                e = wk.tile([128, TH], f32, tag="e")
                nc.vector.scalar_tensor_tensor(
                    e[:], z[:], 0.2, z[:], op0=Alu.mult, op1=Alu.max)
                ex = wk.tile([128, TH], f32, tag="ex")
                nc.scalar.activation(ex[:], e[:], Act.Exp)
                P = wk.tile([128, TH], f32, tag="P")
                nc.vector.tensor_tensor(P[:], ex[:], cnt[:, hs], op=Alu.mult)
                for j in range(4):
                    nc.vector.tensor_scalar(
                        Mv[:, hs, j], P[:], Bm[:, j:j + 1], None, op0=Alu.mult)

            ynT = ps2.tile([128, 4 * T], f32, tag="ynT")
            for i in range(T):
                nc.tensor.matmul(
                    ynT[0:SA, 4 * i:4 * (i + 1)],
                    x[:, SA * i:SA * (i + 1)],
                    M[:, 4 * i:4 * (i + 1)],
                    start=True, stop=True)

            # normalize by denominator (row 64 of y^T)
            ysb = wk.tile([S + 1, GPC], f32, tag="ysb")
            nc.scalar.copy(ysb[:], ynT[0:S + 1, :])
            dn = wk.tile([1, GPC], f32, tag="dn")
            nc.vector.tensor_scalar(
                dn[:], ysb[S:S + 1, :], eps[:], None, op0=Alu.add)
            rp = wk.tile([1, GPC], f32, tag="rp")
            nc.vector.reciprocal_approx_fast(rp[:], dn[:])
            rbc = ps1.tile([S, GPC], f32, tag="mix")
            nc.tensor.matmul(rbc[:], ones64[:], rp[:], start=True, stop=True)
            ynrm = wk.tile([S, GPC], bf16, tag="ynrm")
            nc.vector.tensor_tensor(ynrm[:], ysb[0:S, :], rbc[:], op=Alu.mult)

            # project + bias + sigmoid
            hT = ps1.tile([H, GPC], f32, tag="hT")
            nc.tensor.matmul(hT[:], s["Wb"][:], ynrm[:], start=True, stop=True)
            sg = wk.tile([H, GPC], bf16, tag="sig")
            nc.scalar.activation(sg[:], hT[:], Act.Sigmoid, bias=s["bias"][:])
            sig[p] = sg

        # ---- combine branches + MLP head ----
        prod = wk.tile([H, GPC], bf16, tag="prod")
        nc.vector.tensor_tensor(prod[:], sig["u"][:], sig["d"][:], op=Alu.mult)
        o_ps = ps1.tile([1, GPC], f32, tag="mix")
        nc.tensor.matmul(o_ps[:], mlpW[:], prod[:], start=True, stop=True)
        o_sb = wk.tile([1, GPC], f32, tag="o_sb")
        nc.vector.tensor_scalar(
            o_sb[:], o_ps[:], mlpb[:], None, op0=Alu.add)
        nc.sync.dma_start(out_dram.ap(), o_sb[:])

    nc.compile()
    return nc


def _get_module():
    if "nc" not in _CACHE:
        _CACHE["nc"] = _build_module()
    return _CACHE["nc"]


def _prep_branch(x, ei, W, att_src, att_dst, bias):
    """Host-side sharding + graph-format prep for one branch."""
    import ml_dtypes
    bf = ml_dtypes.bfloat16
    x = np.asarray(x, np.float32)
    src = np.asarray(ei[0]).astype(np.int64)
    dst = np.asarray(ei[1]).astype(np.int64)
    W = np.asarray(W, np.float32)
    w_src = (W @ np.asarray(att_src, np.float32)).astype(np.float32)
    w_dst = (W @ np.asarray(att_dst, np.float32)).astype(np.float32)

    valid = (dst % NPG) == (NPG - 1)
    cnt = np.bincount(src[valid], minlength=N).astype(np.float32)

    per_core = []
    for c in range(NC):
        xs = x[c * NPC:(c + 1) * NPC]
        xab = np.zeros((T, 128, SA), np.float32)
        xab[:, :, :S] = xs.reshape(T, 128, S)
        xab[:, :, S] = 1.0
        xab = np.ascontiguousarray(
            xab.transpose(1, 0, 2).reshape(128, T * SA)).astype(bf)
        # xt[64k+s, m] = x[8192k + m, s]
        xtv = xs.reshape(2, NPC // 2, S).transpose(0, 2, 1)
        xtv = np.ascontiguousarray(xtv.reshape(128, NPC // 2)).astype(bf)
        cnt_t = np.ascontiguousarray(
            cnt[c * NPC:(c + 1) * NPC].reshape(T, 128).T)
        xlast = np.ascontiguousarray(
            xs[NPG - 1::NPG].reshape(128, 4 * S)).astype(bf)
        per_core.append({"xab": xab, "xt": xtv, "cnt": cnt_t, "xlast": xlast})

    wv4 = np.zeros((128, 4), np.float32)
    wv4[:S, 0] = w_src
    wv4[:S, 1] = w_dst
    wv4[S:, 2] = w_src
    wv4[S:, 3] = w_dst
    wdst_rep = np.broadcast_to(w_dst, (128, 4, S)).reshape(128, 4 * S)
    shared = {
        "wv4": wv4.astype(bf),
        "wdst": wdst_rep.astype(bf),
        "W": W.astype(bf),
        "bias": np.asarray(bias, np.float32).reshape(H, 1),
    }
    return per_core, shared


def _build_in_maps(inputs):
    import ml_dtypes
    bf = ml_dtypes.bfloat16
    pcs = {}
    shareds = {}
    pcs["u"], shareds["u"] = _prep_branch(
        inputs["up_x"], inputs["up_edge_index"], inputs["up_W"],
        inputs["up_att_src"], inputs["up_att_dst"], inputs["up_bias"])
    pcs["d"], shareds["d"] = _prep_branch(
        inputs["down_x"], inputs["down_edge_index"], inputs["down_W"],
        inputs["down_att_src"], inputs["down_att_dst"], inputs["down_bias"])

    pp = np.arange(128)
    Qm = np.zeros((4, 128), np.float32)
    Qm[pp // 32, pp] = 1.0
    Bm = np.zeros((128, 4), np.float32)
    Bm[pp, pp // 32] = 1.0

    common = {
        "Qm": Qm.astype(bf),
        "Bm": Bm,
        "ones64": np.ones((1, S), np.float32),
        "ident": np.eye(128, dtype=np.float32),
        "mlpW": np.asarray(inputs["mlp_W"], np.float32).reshape(H, 1).astype(bf),
        "mlpb": np.asarray(inputs["mlp_b"], np.float32).reshape(1, 1),
        "eps": np.full((1, 1), 1e-16, np.float32),
    }
    for p in ("u", "d"):
        for k, v in shareds[p].items():
            common[f"{p}_{k}"] = v

    in_maps = []
    for c in range(NC):
        m = dict(common)
        for p in ("u", "d"):
            for k, v in pcs[p][c].items():
                m[f"{p}_{k}"] = v
        in_maps.append(m)
    return in_maps


def kernel(**inputs):
    from concourse.bass_utils import run_bass_kernel_spmd

    nc = _get_module()
    in_maps = _build_in_maps(inputs)
    res = run_bass_kernel_spmd(nc, in_maps, core_ids=list(range(NC)))
    out = np.concatenate(
        [np.asarray(r["out"], np.float32).reshape(GPC) for r in res.results])
    return out.reshape(G, 1)
